# revision 11
# baseline (speedup 1.0000x reference)
"""Trainium2 Bass kernel for the char-CNN NLP model (data-parallel over 8 cores).

Pipeline:
  host:   emb = x @ emb_w (one-hot projection), laid out [cin, batch, seq],
          quantized to fp8e4 (scaled x64; TRN FP8_EXP4 == ml_dtypes.float8_e4m3)
  device: 3 parallel 1-D conv banks (k=2,3,4; 256 filters each) as fp8
          DoubleRow matmuls (two cin-chunks contracted per pass, fp32 PSUM);
          per (channel, batch) max over sequence; per channel sum of squares
          -> tiny stats tensor per core
  host:   batchnorm statistics from the factorized mean + device sumsq,
          monotone-affine BN+ReLU+maxpool reconstruction from max (min when
          some bn gamma < 0), fc1 -> bn -> relu -> fc2 -> softmax

BN(c+bias) is affine per channel, so max_t relu(bn(c)) = relu(s*M + t) with
M = max_t c if s>=0 else min_t c - exact, and the conv bias cancels inside BN.

Layout trick: each batch's sequence is stored at stride 128 (= S) with no
per-batch gap, so a conv tap at offset kk is one flat contiguous 512-wide
moving operand covering 4 batches; output columns t in [L, 128) accumulate
garbage that the evacuation slices away.

Schedule: quad-major accumulation (each PSUM group stops after one pass over
the weight tiles) so evacuations overlap the next quad's matmul stream. The
two last groups end with single-batch pieces whose stats funnel into one
shared tile and a single tiny trailing DMA.
"""

import os
import numpy as np
import ml_dtypes

# ---------------- problem constants (hardcoded per contract) ----------------
B, S, W, V, E = 128, 128, 16, 128, 32
FILTERS = [256, 256, 256]
KS = [2, 3, 4]
NCLS = 10
EPS = 1e-5
NCORES = 8
BL = B // NCORES             # 16 batches per core
CIN = W * E                  # 512 conv input channels
NCC = CIN // 128             # 4 contraction chunks
NPAIR = NCC // 2             # 2 DoubleRow chunk pairs
LS = [S - k + 1 for k in KS]  # 127, 126, 125 valid conv positions
XH = 8 * 128                 # one batch-half (8 batches x 128) elems
XHP = XH + 32                # padded half stride (tap reads may run 3 past)
EMB_FREE = 2 * 2 * XHP       # (h, c, x) layout per pair tile = 4224
SC_A = 64.0                  # activation fp8 scale
SC_W = 64.0                  # weight fp8 scale
GROUPS = [(0, 0), (1, 0), (1, 1), (2, 0), (2, 1), (0, 1)]
# per-group evacuation pieces: (stat block col, nb batches). The last two
# emitted groups split their final quad 2+1+1 so only single-batch pieces
# trail the matmul stream.
PIECES_FULL = [(0, 4), (5, 4), (10, 4), (15, 4)]
PIECES_BULK = [(0, 4), (5, 4), (10, 4), (15, 2), (18, 1)]
STW = 20                     # bulk stat block width per group (max+sq blocks)
# flat DRAM stats layout: [0:120) 6x20 bulk blocks; [120:124) final cols
# [g5f_max, g5f_sq, g4f_max, g4f_sq]; min variant appends 2 final-min cols
# then per-group min regions
F8 = ml_dtypes.float8_e4m3   # TRN FP8_EXP4: bias 7, max +-240

_CACHE = {}
_LAST_RESULTS = None


def _group_tiles(bank):
    return [(ccp, kk) for ccp in range(NPAIR) for kk in range(KS[bank])]


def _weight_tile_count():
    return sum(len(_group_tiles(bank)) for bank, _ in GROUPS)


def _stats_ncols(need_min):
    # finals region [120:129): g5f_max, g5f_sq, g4f_max, g4f bn_stats x6
    if not need_min:
        return 129
    return 131 + 4 * 16 + 2 * 15  # +2 final mins, then per-group min regions


def _min_base(g):
    return 131 + 16 * g if g < 4 else 195 + 15 * (g - 4)


def _build_bass(need_min):
    import concourse.tile as tile
    from concourse import bacc, mybir
    from contextlib import ExitStack

    nc = bacc.Bacc("TRN2", target_bir_lowering=False, debug=False, enable_asserts=False)

    ntiles = _weight_tile_count()  # 36 DoubleRow tiles of [128, 2, 128]
    nstat = _stats_ncols(need_min)
    DR = mybir.MatmulPerfMode.DoubleRow
    emb_d = nc.dram_tensor(
        "emb", [NPAIR, 128, EMB_FREE], mybir.dt.float8e4, kind="ExternalInput"
    ).ap()
    wts_d = nc.dram_tensor(
        "wts", [128, ntiles * 256], mybir.dt.float8e4, kind="ExternalInput"
    ).ap()
    stats_d = nc.dram_tensor(
        "stats", [128, nstat], mybir.dt.float32, kind="ExternalOutput"
    ).ap()

    with tile.TileContext(nc) as tc, ExitStack() as ctx:
        const_pool = ctx.enter_context(tc.tile_pool(name="const", bufs=1))
        psum_pool = ctx.enter_context(tc.tile_pool(name="psum", bufs=8, space="PSUM"))
        stats_pool = ctx.enter_context(tc.tile_pool(name="stats", bufs=7))
        scr_pool = ctx.enter_context(tc.tile_pool(name="scr", bufs=4))

        # ---- PE warmup: junk DoubleRow matmuls on a zeroed tile while input
        # DMAs stream, so HAM un-throttles before the real stream starts ----
        warm = const_pool.tile([128, 1024], mybir.dt.float8e4, name="warm")
        nc.gpsimd.memset(warm[:], 0.0)
        wlhs = warm[:, :256].rearrange("p (c f) -> p c f", c=2)
        wrhs = warm[:].rearrange("p (c x) -> p c x", c=2)
        wpsum = psum_pool.tile([128, 512], mybir.dt.float32, tag="ps", name="wps")
        for _ in range(3):
            nc.tensor.matmul(
                wpsum[:], wlhs, wrhs, start=True, stop=True, perf_mode=DR
            )

        # ---- load inputs over BOTH HWDGE queues (sync + scalar) so the
        # dispatch serialization halves and the first pieces land sooner ----
        bases = []
        base = 0
        for bank, fc in GROUPS:
            bases.append(base)
            base += len(_group_tiles(bank))
        wt_sb = [
            const_pool.tile(
                [128, len(_group_tiles(GROUPS[g][0])) * 256],
                mybir.dt.float8e4, tag=f"w{g}", name=f"w{g}",
            )
            for g in range(len(GROUPS))
        ]
        emb_sb = [
            const_pool.tile(
                [128, EMB_FREE], mybir.dt.float8e4, tag=f"e{p}", name=f"e{p}"
            )
            for p in range(NPAIR)
        ]

        def load_wt(eng, g, t0, t1):
            eng.dma_start(
                wt_sb[g][:, t0 * 256 : t1 * 256],
                wts_d[:, (bases[g] + t0) * 256 : (bases[g] + t1) * 256],
            )

        def load_emb_piece(eng, p, h, c):
            o = h * 2 * XHP + c * XHP
            eng.dma_start(emb_sb[p][:, o : o + XHP], emb_d[p][:, o : o + XHP])

        # consumption order for quad-major group 0; fine (h,c) pieces so the
        # first matmul unblocks on ~300KB
        nt0 = len(_group_tiles(GROUPS[0][0]))
        load_wt(nc.sync, 0, 0, 1)
        load_emb_piece(nc.scalar, 0, 0, 0)
        load_emb_piece(nc.sync, 0, 0, 1)
        load_emb_piece(nc.scalar, 1, 0, 0)
        load_wt(nc.sync, 0, 1, nt0)
        load_emb_piece(nc.scalar, 1, 0, 1)
        load_emb_piece(nc.sync, 0, 1, 0)
        load_emb_piece(nc.scalar, 1, 1, 0)
        load_emb_piece(nc.sync, 0, 1, 1)
        load_emb_piece(nc.scalar, 1, 1, 1)
        for g, eng in zip(range(1, len(GROUPS)),
                          [nc.sync, nc.scalar, nc.sync, nc.scalar, nc.sync]):
            load_wt(eng, g, 0, len(_group_tiles(GROUPS[g][0])))

        def rhs_ap(ccp, q, kk, boff, nb):
            src = emb_sb[ccp][:].rearrange("p (h c x) -> p h c x", c=2, x=XHP)
            x0 = (q % 2) * 512 + boff * 128 + kk
            return src[:, q // 2, :, x0 : x0 + nb * 128]

        st_sb = [
            stats_pool.tile(
                [128, 36 if need_min else STW], mybir.dt.float32,
                tag=f"st{g}", name=f"st{g}",
            )
            for g in range(len(GROUPS))
        ]
        stf = stats_pool.tile(
            [128, 11 if need_min else 9], mybir.dt.float32, tag="stf", name="stf"
        )

        def run_mms(g, bank, q, boff, nb, pi):
            tiles = _group_tiles(bank)
            wt = wt_sb[g]
            ps = psum_pool.tile(
                [128, nb * 128], mybir.dt.float32, tag="ps", name=f"ps{g}_{pi}"
            )
            for i, (ccp, kk) in enumerate(tiles):
                lhs = wt[:, i * 256 : (i + 1) * 256].rearrange(
                    "p (c f) -> p c f", c=2
                )
                nc.tensor.matmul(
                    ps[:], lhs, rhs_ap(ccp, q, kk, boff, nb),
                    start=(i == 0), stop=(i == len(tiles) - 1), perf_mode=DR,
                )
            return ps

        def emit_piece(g, col, nb, bidx):
            bank, _ = GROUPS[g]
            L = LS[bank]
            ps = run_mms(g, bank, bidx // 4, bidx % 4, nb, f"p{bidx}")
            st = st_sb[g]
            pv = ps[:].rearrange("p (b t) -> p b t", t=128)[:, :, :L]
            nc.vector.tensor_reduce(
                st[:, col : col + nb], pv, axis=mybir.AxisListType.X,
                op=mybir.AluOpType.max,
            )
            if need_min:
                nc.vector.tensor_reduce(
                    st[:, STW + bidx : STW + bidx + nb], pv,
                    axis=mybir.AxisListType.X, op=mybir.AluOpType.min,
                )
            scr = scr_pool.tile([128, 512], mybir.dt.float32, tag="scr")
            scr_v = scr[:, : nb * L].rearrange("p (b t) -> p b t", t=L)
            nc.scalar.activation(
                scr_v, pv,
                mybir.ActivationFunctionType.Square,
                accum_out=st[:, col + nb : col + nb + 1],
            )

        def emit_final(g, fbase, sq_on_act):
            # single-batch piece (bidx 15): max+sq straight into the shared
            # final tile; square on ACT for one group, DVE for the other so
            # the two trailing evacuations run concurrently
            bank, _ = GROUPS[g]
            L = LS[bank]
            ps = run_mms(g, bank, 3, 3, 1, "fin")
            pv = ps[:].rearrange("p (b t) -> p b t", t=128)[:, :, :L]
            nc.vector.tensor_reduce(
                stf[:, fbase : fbase + 1], pv, axis=mybir.AxisListType.X,
                op=mybir.AluOpType.max,
            )
            if need_min:
                mcol = 9 + (0 if g == GROUPS_FIN[0] else 1)
                nc.vector.tensor_reduce(
                    stf[:, mcol : mcol + 1], pv, axis=mybir.AxisListType.X,
                    op=mybir.AluOpType.min,
                )
            if sq_on_act:
                scr = scr_pool.tile([128, 512], mybir.dt.float32, tag="scr")
                scr_v = scr[:, :L].rearrange("p (b t) -> p b t", t=L)
                nc.scalar.activation(
                    scr_v, pv,
                    mybir.ActivationFunctionType.Square,
                    accum_out=stf[:, fbase + 1 : fbase + 2],
                )
            else:
                # DVE one-shot: sumsq recovered on host from count/mean/var
                # of the even/odd interleaves
                nc.vector.bn_stats(stf[:, fbase + 1 : fbase + 7], pv[:, 0, :])

        def dma_bulk(g):
            nc.sync.dma_start(
                stats_d[:, STW * g : STW * g + STW], st_sb[g][:, 0:STW]
            )
            if need_min:
                nmin = 16 if g < 4 else 15
                nc.sync.dma_start(
                    stats_d[:, _min_base(g) : _min_base(g) + nmin],
                    st_sb[g][:, STW : STW + nmin],
                )

        GROUPS_FIN = (5, 4)  # emission order of the two final pieces
        for g in range(4):
            bidx = 0
            for col, nb in PIECES_FULL:
                emit_piece(g, col, nb, bidx)
                bidx += nb
            dma_bulk(g)
        for g in (4, 5):
            bidx = 0
            for col, nb in PIECES_BULK[:3]:
                emit_piece(g, col, nb, bidx)
                bidx += nb
        for g in (4, 5):
            bidx = 12
            for col, nb in PIECES_BULK[3:]:
                emit_piece(g, col, nb, bidx)
                bidx += nb
            dma_bulk(g)
        emit_final(GROUPS_FIN[0], 0, sq_on_act=True)
        emit_final(GROUPS_FIN[1], 2, sq_on_act=False)
        nc.sync.dma_start(
            stats_d[:, 120 : 131 if need_min else 129],
            stf[:, 0 : 11 if need_min else 9],
        )

    nc.compile()
    return nc


def _get_compiled(need_min):
    key = ("nc", need_min)
    if key not in _CACHE:
        _CACHE[key] = _build_bass(need_min)
    return _CACHE[key]


def _maybe_enable_trace():
    if os.environ.get("KERNEL_TRACE") != "1":
        return False
    try:
        import sys, types

        if "antenv.axon_hooks" not in sys.modules:
            mod = types.ModuleType("antenv.axon_hooks")
            _h = {"hook": None}
            mod.set_axon_ntff_profile_hook = lambda h: _h.__setitem__("hook", h)
            mod.get_axon_ntff_profile_hook = lambda: _h["hook"]
            sys.modules["antenv.axon_hooks"] = mod
            import antenv

            antenv.axon_hooks = mod
            from trn_agent_boot.trn_boot import _ntff_profile_via_ctypes

            mod.set_axon_ntff_profile_hook(
                _ntff_profile_via_ctypes("/opt/axon/libaxon_pjrt.so")
            )
        import concourse.bass_utils as bu

        bu.upload_artifacts = lambda tmpdir: tmpdir
        return True
    except Exception:
        return False


def _q8(a, sc):
    return np.clip(np.asarray(a, dtype=np.float32) * sc, -240.0, 240.0).astype(F8)


def kernel(
    x, emb_w,
    conv_w0, conv_b0, bn_g0, bn_b0,
    conv_w1, conv_b1, bn_g1, bn_b1,
    conv_w2, conv_b2, bn_g2, bn_b2,
    fc1_w, fc1_b, bn1_g, bn1_b, fc2_w, fc2_b,
):
    global _LAST_RESULTS
    from concourse.bass_utils import run_bass_kernel_spmd

    x = np.asarray(x, dtype=np.float32)
    emb_w = np.asarray(emb_w, dtype=np.float32)
    conv_ws = [np.asarray(w, dtype=np.float32) for w in (conv_w0, conv_w1, conv_w2)]
    bn_gs = [np.asarray(v, dtype=np.float64) for v in (bn_g0, bn_g1, bn_g2)]
    bn_bs = [np.asarray(v, dtype=np.float64) for v in (bn_b0, bn_b1, bn_b2)]
    need_min = bool((np.concatenate(bn_gs) < 0.0).any())

    # ---- host: embedding (x is one-hot in practice; dense matmul is exact) ----
    e = x.reshape(-1, V) @ emb_w                       # [B*S*W, E]
    e = e.reshape(B, S, CIN)                           # [B, S, 512]
    embT = np.ascontiguousarray(e.transpose(2, 0, 1))  # [512, B, S]
    emb8 = _q8(embT, SC_A)                             # [512, B, 128]

    # ---- pack device inputs ----
    ntiles = _weight_tile_count()
    wts = np.empty((128, ntiles * 256), dtype=F8)
    i = 0
    for bank, fc in GROUPS:
        cwq = _q8(conv_ws[bank], SC_W)                 # [256, 512, k]
        for ccp, kk in _group_tiles(bank):
            blk = cwq[fc * 128 : (fc + 1) * 128,
                      2 * ccp * 128 : (2 * ccp + 2) * 128, kk]  # [f, 2*128]
            # target [p, c*128 + f] = blk[f, c*128 + p]
            wts[:, i * 256 : (i + 1) * 256] = (
                blk.reshape(128, 2, 128).transpose(2, 1, 0).reshape(128, 256)
            )
            i += 1

    # emb8 viewed [pair, c, p, batch, t]
    ev = emb8.reshape(NPAIR, 2, 128, B, S)
    in_maps = []
    for c in range(NCORES):
        v = ev[:, :, :, c * BL : (c + 1) * BL, :].reshape(NPAIR, 2, 128, 2, 8, S)
        tmp = np.zeros((NPAIR, 128, 2, 2, XHP), dtype=F8)
        # [pair, c2, p, h, b, t] -> [pair, p, h, c2, (b t)]
        tmp[:, :, :, :, :XH] = v.transpose(0, 2, 3, 1, 4, 5).reshape(
            NPAIR, 128, 2, 2, XH
        )
        in_maps.append({"emb": tmp.reshape(NPAIR, 128, EMB_FREE), "wts": wts})

    nc = _get_compiled(need_min)
    trace = _maybe_enable_trace()
    res = run_bass_kernel_spmd(
        nc, in_maps, core_ids=list(range(NCORES)), trace=trace,
        tmpdir=os.environ.get("KERNEL_TRACE_DIR") or None,
    )
    _LAST_RESULTS = res

    # ---- host: combine stats -> BN -> pooled -> fc head (float64) ----
    FT = sum(FILTERS)  # 768
    inv = 1.0 / (SC_A * SC_W)
    cmax = np.empty((FT, B), dtype=np.float64)
    cmin = np.empty((FT, B), dtype=np.float64) if need_min else None
    sumsq = np.zeros(FT, dtype=np.float64)
    fin_cols = {5: (120, 121, 129), 4: (122, None, 130)}  # max, sq, min
    for c in range(NCORES):
        stats = res.results[c]["stats"].astype(np.float64)  # [128, nstat]
        for g, (bank, fc) in enumerate(GROUPS):
            ch = bank * 256 + fc * 128
            sl = slice(ch, ch + 128)
            pieces = PIECES_FULL if g < 4 else PIECES_BULK
            bidx = 0
            for col, nb in pieces:
                bs = slice(c * BL + bidx, c * BL + bidx + nb)
                cmax[sl, bs] = stats[:, STW * g + col : STW * g + col + nb] * inv
                sumsq[sl] += stats[:, STW * g + col + nb] * inv * inv
                if need_min:
                    mb = _min_base(g)
                    cmin[sl, bs] = stats[:, mb + bidx : mb + bidx + nb] * inv
                bidx += nb
            if g >= 4:  # final single-batch piece
                fmax, fsq, fmin = fin_cols[g]
                bs = slice(c * BL + 15, c * BL + 16)
                cmax[sl, bs] = stats[:, fmax : fmax + 1] * inv
                if fsq is not None:
                    sumsq[sl] += stats[:, fsq] * inv * inv
                else:  # bn_stats at cols 123..128: [cnt,mean,cnt*var] x even/odd
                    bn = stats[:, 123:129]
                    sq = (bn[:, 2] + bn[:, 0] * bn[:, 1] ** 2
                          + bn[:, 5] + bn[:, 3] * bn[:, 4] ** 2)
                    sumsq[sl] += sq * inv * inv
                if need_min:
                    cmin[sl, bs] = stats[:, fmin : fmin + 1] * inv

    # channel means via the factorized sum (exact: sum_t conv = w . window-sums)
    embT64 = embT.astype(np.float64)
    st_sum = embT64.sum(axis=1)                        # [512, S] summed over batch
    cum = np.concatenate(
        [np.zeros((CIN, 1)), np.cumsum(st_sum, axis=1)], axis=1
    )                                                  # [512, S+1]
    mean = np.empty(FT, dtype=np.float64)
    for bank in range(3):
        k, L = KS[bank], LS[bank]
        cw = conv_ws[bank].astype(np.float64)          # [256, 512, k]
        hs = np.stack([cum[:, kk + L] - cum[:, kk] for kk in range(k)], axis=1)
        mean[bank * 256 : (bank + 1) * 256] = (
            np.einsum("fck,ck->f", cw, hs) / (B * L)
        )

    counts = np.repeat([B * L for L in LS], FILTERS)
    var = sumsq / counts - mean * mean
    g_all = np.concatenate(bn_gs)
    b_all = np.concatenate(bn_bs)
    s = g_all / np.sqrt(var + EPS)
    shift = b_all - mean * s
    M = np.where(s[:, None] >= 0.0, cmax, cmin if need_min else cmax)  # [768, B]
    pooled = np.maximum(s[:, None] * M + shift[:, None], 0.0).T  # [B, 768]

    z = pooled @ np.asarray(fc1_w, dtype=np.float64) + np.asarray(
        fc1_b, dtype=np.float64
    )
    mu = z.mean(axis=0, keepdims=True)
    vz = np.square(z - mu).mean(axis=0, keepdims=True)
    z = (z - mu) / np.sqrt(vz + EPS) * np.asarray(
        bn1_g, dtype=np.float64
    ) + np.asarray(bn1_b, dtype=np.float64)
    z = np.maximum(z, 0.0)
    logits = z @ np.asarray(fc2_w, dtype=np.float64) + np.asarray(
        fc2_b, dtype=np.float64
    )
    logits -= logits.max(axis=1, keepdims=True)
    p = np.exp(logits)
    p /= p.sum(axis=1, keepdims=True)
    return p.astype(np.float32)


# revision 14
# speedup vs baseline: 1.0051x; 1.0051x over previous
"""Trainium2 Bass kernel for the char-CNN NLP model (data-parallel over 8 cores).

Pipeline:
  host:   emb = x @ emb_w (one-hot projection), laid out [cin, batch, seq],
          quantized to fp8e4 (scaled x64; TRN FP8_EXP4 == ml_dtypes.float8_e4m3)
  device: 3 parallel 1-D conv banks (k=2,3,4; 256 filters each) as fp8
          DoubleRow matmuls (two cin-chunks contracted per pass, fp32 PSUM);
          per (channel, batch) max over sequence; per channel sum of squares
          -> tiny stats tensor per core
  host:   batchnorm statistics from the factorized mean + device sumsq,
          monotone-affine BN+ReLU+maxpool reconstruction from max (min when
          some bn gamma < 0), fc1 -> bn -> relu -> fc2 -> softmax

BN(c+bias) is affine per channel, so max_t relu(bn(c)) = relu(s*M + t) with
M = max_t c if s>=0 else min_t c - exact, and the conv bias cancels inside BN.

Layout trick: each batch's sequence is stored at stride 128 (= S) with no
per-batch gap, so a conv tap at offset kk is one flat contiguous 512-wide
moving operand covering 4 batches; output columns t in [L, 128) accumulate
garbage that the evacuation slices away.

Schedule: quad-major accumulation (each PSUM group stops after one pass over
the weight tiles) so evacuations overlap the next quad's matmul stream. The
two last groups end with single-batch pieces whose stats funnel into one
shared tile and a single tiny trailing DMA.
"""

import os
import numpy as np
import ml_dtypes

# ---------------- problem constants (hardcoded per contract) ----------------
B, S, W, V, E = 128, 128, 16, 128, 32
FILTERS = [256, 256, 256]
KS = [2, 3, 4]
NCLS = 10
EPS = 1e-5
NCORES = 8
BL = B // NCORES             # 16 batches per core
CIN = W * E                  # 512 conv input channels
NCC = CIN // 128             # 4 contraction chunks
NPAIR = NCC // 2             # 2 DoubleRow chunk pairs
LS = [S - k + 1 for k in KS]  # 127, 126, 125 valid conv positions
XH = 8 * 128                 # one batch-half (8 batches x 128) elems
XHP = XH + 32                # padded half stride (tap reads may run 3 past)
EMB_FREE = 2 * 2 * XHP       # (h, c, x) layout per pair tile = 4224
SC_A = 64.0                  # activation fp8 scale
SC_W = 64.0                  # weight fp8 scale
GROUPS = [(0, 0), (1, 0), (1, 1), (2, 0), (2, 1), (0, 1)]
# per-group evacuation pieces: (stat block col, nb batches). The last two
# emitted groups split their final quad 2+1+1 so only single-batch pieces
# trail the matmul stream.
PIECES_FULL = [(0, 4), (5, 4), (10, 4), (15, 4)]
PIECES_BULK = [(0, 4), (5, 4), (10, 4), (15, 2), (18, 1)]
STW = 20                     # bulk stat block width per group (max+sq blocks)
# flat DRAM stats layout: [0:120) 6x20 bulk blocks; [120:124) final cols
# [g5f_max, g5f_sq, g4f_max, g4f_sq]; min variant appends 2 final-min cols
# then per-group min regions
F8 = ml_dtypes.float8_e4m3   # TRN FP8_EXP4: bias 7, max +-240

_CACHE = {}
_LAST_RESULTS = None


def _group_tiles(bank):
    return [(ccp, kk) for ccp in range(NPAIR) for kk in range(KS[bank])]


def _weight_tile_count():
    return sum(len(_group_tiles(bank)) for bank, _ in GROUPS)


def _stats_ncols(need_min):
    # finals region [120:127): g5f_max, g5f bn_stats x6 (+1 min col when used)
    if not need_min:
        return 127
    return 128 + 5 * 16 + 15  # per-group min regions after the finals


def _min_base(g):
    return 128 + 16 * g if g < 5 else 208


def _build_bass(need_min):
    import concourse.tile as tile
    from concourse import bacc, mybir
    from contextlib import ExitStack

    nc = bacc.Bacc("TRN2", target_bir_lowering=False, debug=False, enable_asserts=False)

    ntiles = _weight_tile_count()  # 36 DoubleRow tiles of [128, 2, 128]
    nstat = _stats_ncols(need_min)
    DR = mybir.MatmulPerfMode.DoubleRow
    emb_d = nc.dram_tensor(
        "emb", [NPAIR, 128, EMB_FREE], mybir.dt.float8e4, kind="ExternalInput"
    ).ap()
    wts_d = nc.dram_tensor(
        "wts", [128, ntiles * 256], mybir.dt.float8e4, kind="ExternalInput"
    ).ap()
    stats_d = nc.dram_tensor(
        "stats", [128, nstat], mybir.dt.float32, kind="ExternalOutput"
    ).ap()

    with tile.TileContext(nc) as tc, ExitStack() as ctx:
        const_pool = ctx.enter_context(tc.tile_pool(name="const", bufs=1))
        psum_pool = ctx.enter_context(tc.tile_pool(name="psum", bufs=8, space="PSUM"))
        stats_pool = ctx.enter_context(tc.tile_pool(name="stats", bufs=7))
        scr_pool = ctx.enter_context(tc.tile_pool(name="scr", bufs=4))

        # ---- PE warmup: junk DoubleRow matmuls on a zeroed tile while input
        # DMAs stream, so HAM un-throttles before the real stream starts ----
        warm = const_pool.tile([128, 1024], mybir.dt.float8e4, name="warm")
        nc.gpsimd.memset(warm[:], 0.0)
        wlhs = warm[:, :256].rearrange("p (c f) -> p c f", c=2)
        wrhs = warm[:].rearrange("p (c x) -> p c x", c=2)
        wpsum = psum_pool.tile([128, 512], mybir.dt.float32, tag="ps", name="wps")
        for _ in range(3):
            nc.tensor.matmul(
                wpsum[:], wlhs, wrhs, start=True, stop=True, perf_mode=DR
            )

        # ---- load inputs over BOTH HWDGE queues (sync + scalar) so the
        # dispatch serialization halves and the first pieces land sooner ----
        bases = []
        base = 0
        for bank, fc in GROUPS:
            bases.append(base)
            base += len(_group_tiles(bank))
        wt_sb = [
            const_pool.tile(
                [128, len(_group_tiles(GROUPS[g][0])) * 256],
                mybir.dt.float8e4, tag=f"w{g}", name=f"w{g}",
            )
            for g in range(len(GROUPS))
        ]
        emb_sb = [
            const_pool.tile(
                [128, EMB_FREE], mybir.dt.float8e4, tag=f"e{p}", name=f"e{p}"
            )
            for p in range(NPAIR)
        ]

        def load_wt(eng, g, t0, t1):
            eng.dma_start(
                wt_sb[g][:, t0 * 256 : t1 * 256],
                wts_d[:, (bases[g] + t0) * 256 : (bases[g] + t1) * 256],
            )

        def load_emb_piece(eng, p, h, c):
            o = h * 2 * XHP + c * XHP
            eng.dma_start(emb_sb[p][:, o : o + XHP], emb_d[p][:, o : o + XHP])

        # group 0 accumulates pair0's taps first (PSUM groups stay open),
        # then pair1's, so the matmul stream starts on ~370KB and consumes
        # at the DMA arrival rate
        nt0 = len(_group_tiles(GROUPS[0][0]))
        load_wt(nc.sync, 0, 0, nt0 // 2)          # pair0 taps of group 0
        load_emb_piece(nc.scalar, 0, 0, 0)
        load_emb_piece(nc.sync, 0, 0, 1)
        load_emb_piece(nc.scalar, 0, 1, 1)
        load_emb_piece(nc.sync, 0, 1, 0)
        load_emb_piece(nc.scalar, 1, 0, 0)
        load_wt(nc.sync, 0, nt0 // 2, nt0)        # pair1 taps of group 0
        load_emb_piece(nc.scalar, 1, 0, 1)
        load_emb_piece(nc.scalar, 1, 1, 0)
        load_wt(nc.sync, 1, 0, len(_group_tiles(GROUPS[1][0])))
        load_emb_piece(nc.scalar, 1, 1, 1)
        for g, eng in zip(range(2, len(GROUPS)),
                          [nc.scalar, nc.sync, nc.scalar, nc.sync]):
            load_wt(eng, g, 0, len(_group_tiles(GROUPS[g][0])))

        def rhs_ap(ccp, q, kk, boff, nb):
            src = emb_sb[ccp][:].rearrange("p (h c x) -> p h c x", c=2, x=XHP)
            x0 = (q % 2) * 512 + boff * 128 + kk
            return src[:, q // 2, :, x0 : x0 + nb * 128]

        st_sb = [
            stats_pool.tile(
                [128, 36 if need_min else STW], mybir.dt.float32,
                tag=f"st{g}", name=f"st{g}",
            )
            for g in range(len(GROUPS))
        ]
        stf = stats_pool.tile(
            [128, 8 if need_min else 7], mybir.dt.float32, tag="stf", name="stf"
        )

        def run_mms(g, bank, q, boff, nb, pi):
            tiles = _group_tiles(bank)
            wt = wt_sb[g]
            ps = psum_pool.tile(
                [128, nb * 128], mybir.dt.float32, tag="ps", name=f"ps{g}_{pi}"
            )
            for i, (ccp, kk) in enumerate(tiles):
                lhs = wt[:, i * 256 : (i + 1) * 256].rearrange(
                    "p (c f) -> p c f", c=2
                )
                nc.tensor.matmul(
                    ps[:], lhs, rhs_ap(ccp, q, kk, boff, nb),
                    start=(i == 0), stop=(i == len(tiles) - 1), perf_mode=DR,
                )
            return ps

        def emit_piece(g, col, nb, bidx):
            bank, _ = GROUPS[g]
            L = LS[bank]
            ps = run_mms(g, bank, bidx // 4, bidx % 4, nb, f"p{bidx}")
            st = st_sb[g]
            pv = ps[:].rearrange("p (b t) -> p b t", t=128)[:, :, :L]
            nc.vector.tensor_reduce(
                st[:, col : col + nb], pv, axis=mybir.AxisListType.X,
                op=mybir.AluOpType.max,
            )
            if need_min:
                nc.vector.tensor_reduce(
                    st[:, STW + bidx : STW + bidx + nb], pv,
                    axis=mybir.AxisListType.X, op=mybir.AluOpType.min,
                )
            scr = scr_pool.tile([128, 512], mybir.dt.float32, tag="scr")
            scr_v = scr[:, : nb * L].rearrange("p (b t) -> p b t", t=L)
            nc.scalar.activation(
                scr_v, pv,
                mybir.ActivationFunctionType.Square,
                accum_out=st[:, col + nb : col + nb + 1],
            )

        def emit_final(g):
            # single-batch piece (bidx 15): max + bn_stats (sumsq recovered
            # on host from count/mean/var of the even/odd interleaves), both
            # on DVE, straight into the shared final tile
            bank, _ = GROUPS[g]
            L = LS[bank]
            ps = run_mms(g, bank, 3, 3, 1, "fin")
            pv = ps[:].rearrange("p (b t) -> p b t", t=128)[:, :, :L]
            nc.vector.tensor_reduce(
                stf[:, 0:1], pv, axis=mybir.AxisListType.X,
                op=mybir.AluOpType.max,
            )
            nc.vector.bn_stats(stf[:, 1:7], pv[:, 0, :])
            if need_min:
                nc.vector.tensor_reduce(
                    stf[:, 7:8], pv, axis=mybir.AxisListType.X,
                    op=mybir.AluOpType.min,
                )

        def dma_bulk(g):
            # bulk stats go out on the idle gpsimd (SWDGE) queue so the sync
            # queue's tail only carries the tiny final-stats DMA
            nc.gpsimd.dma_start(
                stats_d[:, STW * g : STW * g + STW], st_sb[g][:, 0:STW]
            )
            if need_min:
                nmin = 16 if g < 5 else 15
                nc.gpsimd.dma_start(
                    stats_d[:, _min_base(g) : _min_base(g) + nmin],
                    st_sb[g][:, STW : STW + nmin],
                )

        # group 0: phased accumulation (pair0 taps as soon as its halves
        # land, pair1 taps later; PSUM groups stay open in between)
        bank0 = GROUPS[0][0]
        t0 = _group_tiles(bank0)
        half = len(t0) // 2
        g0_ps = [
            psum_pool.tile([128, 512], mybir.dt.float32, tag="ps", name=f"ps0_{q}")
            for q in range(4)
        ]
        for phase, qs in ((0, (0, 1)), (0, (2, 3)), (1, (0, 1)), (1, (2, 3))):
            tiles = t0[:half] if phase == 0 else t0[half:]
            for q in qs:
                for j, (ccp, kk) in enumerate(tiles):
                    i = phase * half + j
                    lhs = wt_sb[0][:, i * 256 : (i + 1) * 256].rearrange(
                        "p (c f) -> p c f", c=2
                    )
                    nc.tensor.matmul(
                        g0_ps[q][:], lhs, rhs_ap(ccp, q, kk, 0, 4),
                        start=(phase == 0 and j == 0),
                        stop=(phase == 1 and j == half - 1), perf_mode=DR,
                    )
                if phase == 1:
                    col, nb = PIECES_FULL[q]
                    st = st_sb[0]
                    L = LS[bank0]
                    pv = g0_ps[q][:].rearrange("p (b t) -> p b t", t=128)[:, :, :L]
                    nc.vector.tensor_reduce(
                        st[:, col : col + nb], pv, axis=mybir.AxisListType.X,
                        op=mybir.AluOpType.max,
                    )
                    if need_min:
                        nc.vector.tensor_reduce(
                            st[:, STW + q * 4 : STW + q * 4 + nb], pv,
                            axis=mybir.AxisListType.X, op=mybir.AluOpType.min,
                        )
                    scr = scr_pool.tile([128, 512], mybir.dt.float32, tag="scr")
                    scr_v = scr[:, : nb * L].rearrange("p (b t) -> p b t", t=L)
                    nc.scalar.activation(
                        scr_v, pv,
                        mybir.ActivationFunctionType.Square,
                        accum_out=st[:, col + nb : col + nb + 1],
                    )
        dma_bulk(0)

        for g in range(1, 5):
            bidx = 0
            for col, nb in PIECES_FULL:
                emit_piece(g, col, nb, bidx)
                bidx += nb
            dma_bulk(g)
        bidx = 0
        for col, nb in PIECES_BULK:
            emit_piece(5, col, nb, bidx)
            bidx += nb
        dma_bulk(5)
        emit_final(5)
        nc.sync.dma_start(
            stats_d[:, 120 : 128 if need_min else 127],
            stf[:, 0 : 8 if need_min else 7],
        )

    nc.compile()
    return nc


def _get_compiled(need_min):
    key = ("nc", need_min)
    if key not in _CACHE:
        _CACHE[key] = _build_bass(need_min)
    return _CACHE[key]


def _maybe_enable_trace():
    if os.environ.get("KERNEL_TRACE") != "1":
        return False
    try:
        import sys, types

        if "antenv.axon_hooks" not in sys.modules:
            mod = types.ModuleType("antenv.axon_hooks")
            _h = {"hook": None}
            mod.set_axon_ntff_profile_hook = lambda h: _h.__setitem__("hook", h)
            mod.get_axon_ntff_profile_hook = lambda: _h["hook"]
            sys.modules["antenv.axon_hooks"] = mod
            import antenv

            antenv.axon_hooks = mod
            from trn_agent_boot.trn_boot import _ntff_profile_via_ctypes

            mod.set_axon_ntff_profile_hook(
                _ntff_profile_via_ctypes("/opt/axon/libaxon_pjrt.so")
            )
        import concourse.bass_utils as bu

        bu.upload_artifacts = lambda tmpdir: tmpdir
        return True
    except Exception:
        return False


def _q8(a, sc):
    return np.clip(np.asarray(a, dtype=np.float32) * sc, -240.0, 240.0).astype(F8)


def kernel(
    x, emb_w,
    conv_w0, conv_b0, bn_g0, bn_b0,
    conv_w1, conv_b1, bn_g1, bn_b1,
    conv_w2, conv_b2, bn_g2, bn_b2,
    fc1_w, fc1_b, bn1_g, bn1_b, fc2_w, fc2_b,
):
    global _LAST_RESULTS
    from concourse.bass_utils import run_bass_kernel_spmd

    x = np.asarray(x, dtype=np.float32)
    emb_w = np.asarray(emb_w, dtype=np.float32)
    conv_ws = [np.asarray(w, dtype=np.float32) for w in (conv_w0, conv_w1, conv_w2)]
    bn_gs = [np.asarray(v, dtype=np.float64) for v in (bn_g0, bn_g1, bn_g2)]
    bn_bs = [np.asarray(v, dtype=np.float64) for v in (bn_b0, bn_b1, bn_b2)]
    need_min = bool((np.concatenate(bn_gs) < 0.0).any())

    # ---- host: embedding (x is one-hot in practice; dense matmul is exact) ----
    e = x.reshape(-1, V) @ emb_w                       # [B*S*W, E]
    e = e.reshape(B, S, CIN)                           # [B, S, 512]
    embT = np.ascontiguousarray(e.transpose(2, 0, 1))  # [512, B, S]
    emb8 = _q8(embT, SC_A)                             # [512, B, 128]

    # ---- pack device inputs ----
    ntiles = _weight_tile_count()
    wts = np.empty((128, ntiles * 256), dtype=F8)
    i = 0
    for bank, fc in GROUPS:
        cwq = _q8(conv_ws[bank], SC_W)                 # [256, 512, k]
        for ccp, kk in _group_tiles(bank):
            blk = cwq[fc * 128 : (fc + 1) * 128,
                      2 * ccp * 128 : (2 * ccp + 2) * 128, kk]  # [f, 2*128]
            # target [p, c*128 + f] = blk[f, c*128 + p]
            wts[:, i * 256 : (i + 1) * 256] = (
                blk.reshape(128, 2, 128).transpose(2, 1, 0).reshape(128, 256)
            )
            i += 1

    # emb8 viewed [pair, c, p, batch, t]
    ev = emb8.reshape(NPAIR, 2, 128, B, S)
    in_maps = []
    for c in range(NCORES):
        v = ev[:, :, :, c * BL : (c + 1) * BL, :].reshape(NPAIR, 2, 128, 2, 8, S)
        tmp = np.zeros((NPAIR, 128, 2, 2, XHP), dtype=F8)
        # [pair, c2, p, h, b, t] -> [pair, p, h, c2, (b t)]
        tmp[:, :, :, :, :XH] = v.transpose(0, 2, 3, 1, 4, 5).reshape(
            NPAIR, 128, 2, 2, XH
        )
        in_maps.append({"emb": tmp.reshape(NPAIR, 128, EMB_FREE), "wts": wts})

    nc = _get_compiled(need_min)
    trace = _maybe_enable_trace()
    res = run_bass_kernel_spmd(
        nc, in_maps, core_ids=list(range(NCORES)), trace=trace,
        tmpdir=os.environ.get("KERNEL_TRACE_DIR") or None,
    )
    _LAST_RESULTS = res

    # ---- host: combine stats -> BN -> pooled -> fc head (float64) ----
    FT = sum(FILTERS)  # 768
    inv = 1.0 / (SC_A * SC_W)
    cmax = np.empty((FT, B), dtype=np.float64)
    cmin = np.empty((FT, B), dtype=np.float64) if need_min else None
    sumsq = np.zeros(FT, dtype=np.float64)
    for c in range(NCORES):
        stats = res.results[c]["stats"].astype(np.float64)  # [128, nstat]
        for g, (bank, fc) in enumerate(GROUPS):
            ch = bank * 256 + fc * 128
            sl = slice(ch, ch + 128)
            pieces = PIECES_FULL if g < 5 else PIECES_BULK
            bidx = 0
            for col, nb in pieces:
                bs = slice(c * BL + bidx, c * BL + bidx + nb)
                cmax[sl, bs] = stats[:, STW * g + col : STW * g + col + nb] * inv
                sumsq[sl] += stats[:, STW * g + col + nb] * inv * inv
                if need_min:
                    mb = _min_base(g)
                    cmin[sl, bs] = stats[:, mb + bidx : mb + bidx + nb] * inv
                bidx += nb
            if g == 5:  # final single-batch piece: max + bn_stats
                bs = slice(c * BL + 15, c * BL + 16)
                cmax[sl, bs] = stats[:, 120:121] * inv
                bn = stats[:, 121:127]  # [cnt,mean,cnt*var] x even/odd
                sq = (bn[:, 2] + bn[:, 0] * bn[:, 1] ** 2
                      + bn[:, 5] + bn[:, 3] * bn[:, 4] ** 2)
                sumsq[sl] += sq * inv * inv
                if need_min:
                    cmin[sl, bs] = stats[:, 127:128] * inv

    # channel means via the factorized sum (exact: sum_t conv = w . window-sums)
    embT64 = embT.astype(np.float64)
    st_sum = embT64.sum(axis=1)                        # [512, S] summed over batch
    cum = np.concatenate(
        [np.zeros((CIN, 1)), np.cumsum(st_sum, axis=1)], axis=1
    )                                                  # [512, S+1]
    mean = np.empty(FT, dtype=np.float64)
    for bank in range(3):
        k, L = KS[bank], LS[bank]
        cw = conv_ws[bank].astype(np.float64)          # [256, 512, k]
        hs = np.stack([cum[:, kk + L] - cum[:, kk] for kk in range(k)], axis=1)
        mean[bank * 256 : (bank + 1) * 256] = (
            np.einsum("fck,ck->f", cw, hs) / (B * L)
        )

    counts = np.repeat([B * L for L in LS], FILTERS)
    var = sumsq / counts - mean * mean
    g_all = np.concatenate(bn_gs)
    b_all = np.concatenate(bn_bs)
    s = g_all / np.sqrt(var + EPS)
    shift = b_all - mean * s
    M = np.where(s[:, None] >= 0.0, cmax, cmin if need_min else cmax)  # [768, B]
    pooled = np.maximum(s[:, None] * M + shift[:, None], 0.0).T  # [B, 768]

    z = pooled @ np.asarray(fc1_w, dtype=np.float64) + np.asarray(
        fc1_b, dtype=np.float64
    )
    mu = z.mean(axis=0, keepdims=True)
    vz = np.square(z - mu).mean(axis=0, keepdims=True)
    z = (z - mu) / np.sqrt(vz + EPS) * np.asarray(
        bn1_g, dtype=np.float64
    ) + np.asarray(bn1_b, dtype=np.float64)
    z = np.maximum(z, 0.0)
    logits = z @ np.asarray(fc2_w, dtype=np.float64) + np.asarray(
        fc2_b, dtype=np.float64
    )
    logits -= logits.max(axis=1, keepdims=True)
    p = np.exp(logits)
    p /= p.sum(axis=1, keepdims=True)
    return p.astype(np.float32)


# revision 16
# speedup vs baseline: 1.0233x; 1.0181x over previous
"""Trainium2 Bass kernel for the char-CNN NLP model (data-parallel over 8 cores).

Pipeline:
  host:   emb = x @ emb_w (one-hot projection), laid out [cin, batch, seq],
          quantized to fp8e4 (scaled x64; TRN FP8_EXP4 == ml_dtypes.float8_e4m3)
  device: 3 parallel 1-D conv banks (k=2,3,4; 256 filters each) as fp8
          DoubleRow matmuls (two cin-chunks contracted per pass, fp32 PSUM);
          per (channel, batch) max over sequence; per channel sum of squares
          -> tiny stats tensor per core
  host:   batchnorm statistics from the factorized mean + device sumsq,
          monotone-affine BN+ReLU+maxpool reconstruction from max (min when
          some bn gamma < 0), fc1 -> bn -> relu -> fc2 -> softmax

BN(c+bias) is affine per channel, so max_t relu(bn(c)) = relu(s*M + t) with
M = max_t c if s>=0 else min_t c - exact, and the conv bias cancels inside BN.

Layout trick: each batch's sequence is stored at stride 128 (= S) with no
per-batch gap, so a conv tap at offset kk is one flat contiguous 512-wide
moving operand covering 4 batches; output columns t in [L, 128) accumulate
garbage that the evacuation slices away.

Schedule: quad-major accumulation (each PSUM group stops after one pass over
the weight tiles) so evacuations overlap the next quad's matmul stream. The
two last groups end with single-batch pieces whose stats funnel into one
shared tile and a single tiny trailing DMA.
"""

import os
import numpy as np
import ml_dtypes

# ---------------- problem constants (hardcoded per contract) ----------------
B, S, W, V, E = 128, 128, 16, 128, 32
FILTERS = [256, 256, 256]
KS = [2, 3, 4]
NCLS = 10
EPS = 1e-5
NCORES = 8
BL = B // NCORES             # 16 batches per core
CIN = W * E                  # 512 conv input channels
NCC = CIN // 128             # 4 contraction chunks
NPAIR = NCC // 2             # 2 DoubleRow chunk pairs
LS = [S - k + 1 for k in KS]  # 127, 126, 125 valid conv positions
XH = 8 * 128                 # one batch-half (8 batches x 128) elems
XHP = XH + 32                # padded half stride (tap reads may run 3 past)
EMB_FREE = 2 * 2 * XHP       # (h, c, x) layout per pair tile = 4224
SC_A = 64.0                  # activation fp8 scale
SC_W = 64.0                  # weight fp8 scale
GROUPS = [(0, 0), (1, 0), (1, 1), (2, 0), (2, 1), (0, 1)]
# per-group evacuation pieces: (stat block col, nb batches). The last two
# emitted groups split their final quad 2+1+1 so only single-batch pieces
# trail the matmul stream.
PIECES_FULL = [(0, 4), (5, 4), (10, 4), (15, 4)]
PIECES_BULK = [(0, 4), (5, 4), (10, 4), (15, 2), (18, 1)]
STW = 20                     # bulk stat block width per group (max+sq blocks)
# flat DRAM stats layout: [0:120) 6x20 bulk blocks; [120:124) final cols
# [g5f_max, g5f_sq, g4f_max, g4f_sq]; min variant appends 2 final-min cols
# then per-group min regions
F8 = ml_dtypes.float8_e4m3   # TRN FP8_EXP4: bias 7, max +-240

_CACHE = {}
_LAST_RESULTS = None


def _group_tiles(bank):
    return [(ccp, kk) for ccp in range(NPAIR) for kk in range(KS[bank])]


def _weight_tile_count():
    return sum(len(_group_tiles(bank)) for bank, _ in GROUPS)


def _stats_ncols(need_min):
    # finals region [120:148): g5's last three pieces (2+1+1 batches), each
    # as [max cols, bn_stats 6 cols per batch]; +4 min cols when used
    if not need_min:
        return 148
    return 152 + 5 * 16 + 12  # per-group min regions after the finals


def _min_base(g):
    return 152 + 16 * g if g < 5 else 232


def _build_bass(need_min):
    import concourse.tile as tile
    from concourse import bacc, mybir
    from contextlib import ExitStack

    nc = bacc.Bacc("TRN2", target_bir_lowering=False, debug=False, enable_asserts=False)

    ntiles = _weight_tile_count()  # 36 DoubleRow tiles of [128, 2, 128]
    nstat = _stats_ncols(need_min)
    DR = mybir.MatmulPerfMode.DoubleRow
    emb_d = nc.dram_tensor(
        "emb", [NPAIR, 128, EMB_FREE], mybir.dt.float8e4, kind="ExternalInput"
    ).ap()
    wts_d = nc.dram_tensor(
        "wts", [128, ntiles * 256], mybir.dt.float8e4, kind="ExternalInput"
    ).ap()
    stats_d = nc.dram_tensor(
        "stats", [128, nstat], mybir.dt.float32, kind="ExternalOutput"
    ).ap()

    with tile.TileContext(nc) as tc, ExitStack() as ctx:
        const_pool = ctx.enter_context(tc.tile_pool(name="const", bufs=1))
        psum_pool = ctx.enter_context(tc.tile_pool(name="psum", bufs=8, space="PSUM"))
        stats_pool = ctx.enter_context(tc.tile_pool(name="stats", bufs=7))
        scr_pool = ctx.enter_context(tc.tile_pool(name="scr", bufs=4))

        # ---- PE warmup: junk DoubleRow matmuls on a zeroed tile while input
        # DMAs stream, so HAM un-throttles before the real stream starts ----
        warm = const_pool.tile([128, 1024], mybir.dt.float8e4, name="warm")
        nc.gpsimd.memset(warm[:], 0.0)
        wlhs = warm[:, :256].rearrange("p (c f) -> p c f", c=2)
        wrhs = warm[:].rearrange("p (c x) -> p c x", c=2)
        wpsum = psum_pool.tile([128, 512], mybir.dt.float32, tag="ps", name="wps")
        for _ in range(3):
            nc.tensor.matmul(
                wpsum[:], wlhs, wrhs, start=True, stop=True, perf_mode=DR
            )

        # ---- load inputs over BOTH HWDGE queues (sync + scalar) so the
        # dispatch serialization halves and the first pieces land sooner ----
        bases = []
        base = 0
        for bank, fc in GROUPS:
            bases.append(base)
            base += len(_group_tiles(bank))
        wt_sb = [
            const_pool.tile(
                [128, len(_group_tiles(GROUPS[g][0])) * 256],
                mybir.dt.float8e4, tag=f"w{g}", name=f"w{g}",
            )
            for g in range(len(GROUPS))
        ]
        emb_sb = [
            const_pool.tile(
                [128, EMB_FREE], mybir.dt.float8e4, tag=f"e{p}", name=f"e{p}"
            )
            for p in range(NPAIR)
        ]

        def load_wt(eng, g, t0, t1):
            eng.dma_start(
                wt_sb[g][:, t0 * 256 : t1 * 256],
                wts_d[:, (bases[g] + t0) * 256 : (bases[g] + t1) * 256],
            )

        def load_emb_piece(eng, p, h, c):
            o = h * 2 * XHP + c * XHP
            eng.dma_start(emb_sb[p][:, o : o + XHP], emb_d[p][:, o : o + XHP])

        # group 0 accumulates pair0's taps first (PSUM groups stay open),
        # then pair1's, so the matmul stream starts on ~370KB and consumes
        # at the DMA arrival rate
        nt0 = len(_group_tiles(GROUPS[0][0]))
        load_wt(nc.sync, 0, 0, nt0 // 2)          # pair0 taps of group 0
        load_emb_piece(nc.scalar, 0, 0, 0)
        load_emb_piece(nc.sync, 0, 0, 1)
        load_emb_piece(nc.scalar, 0, 1, 1)
        load_emb_piece(nc.sync, 0, 1, 0)
        load_emb_piece(nc.scalar, 1, 0, 0)
        load_wt(nc.sync, 0, nt0 // 2, nt0)        # pair1 taps of group 0
        load_emb_piece(nc.scalar, 1, 0, 1)
        load_emb_piece(nc.scalar, 1, 1, 0)
        load_wt(nc.sync, 1, 0, len(_group_tiles(GROUPS[1][0])))
        load_emb_piece(nc.scalar, 1, 1, 1)
        for g, eng in zip(range(2, len(GROUPS)),
                          [nc.scalar, nc.sync, nc.scalar, nc.sync]):
            load_wt(eng, g, 0, len(_group_tiles(GROUPS[g][0])))

        def rhs_ap(ccp, q, kk, boff, nb):
            src = emb_sb[ccp][:].rearrange("p (h c x) -> p h c x", c=2, x=XHP)
            x0 = (q % 2) * 512 + boff * 128 + kk
            return src[:, q // 2, :, x0 : x0 + nb * 128]

        st_sb = [
            stats_pool.tile(
                [128, 36 if need_min else STW], mybir.dt.float32,
                tag=f"st{g}", name=f"st{g}",
            )
            for g in range(len(GROUPS))
        ]
        stf = stats_pool.tile(
            [128, 32 if need_min else 28], mybir.dt.float32, tag="stf", name="stf"
        )

        def run_mms(g, bank, q, boff, nb, pi):
            tiles = _group_tiles(bank)
            wt = wt_sb[g]
            ps = psum_pool.tile(
                [128, nb * 128], mybir.dt.float32, tag="ps", name=f"ps{g}_{pi}"
            )
            for i, (ccp, kk) in enumerate(tiles):
                lhs = wt[:, i * 256 : (i + 1) * 256].rearrange(
                    "p (c f) -> p c f", c=2
                )
                nc.tensor.matmul(
                    ps[:], lhs, rhs_ap(ccp, q, kk, boff, nb),
                    start=(i == 0), stop=(i == len(tiles) - 1), perf_mode=DR,
                )
            return ps

        def emit_piece(g, col, nb, bidx):
            bank, _ = GROUPS[g]
            L = LS[bank]
            ps = run_mms(g, bank, bidx // 4, bidx % 4, nb, f"p{bidx}")
            st = st_sb[g]
            pv = ps[:].rearrange("p (b t) -> p b t", t=128)[:, :, :L]
            nc.vector.tensor_reduce(
                st[:, col : col + nb], pv, axis=mybir.AxisListType.X,
                op=mybir.AluOpType.max,
            )
            if need_min:
                nc.vector.tensor_reduce(
                    st[:, STW + bidx : STW + bidx + nb], pv,
                    axis=mybir.AxisListType.X, op=mybir.AluOpType.min,
                )
            scr = scr_pool.tile([128, 512], mybir.dt.float32, tag="scr")
            scr_v = scr[:, : nb * L].rearrange("p (b t) -> p b t", t=L)
            nc.scalar.activation(
                scr_v, pv,
                mybir.ActivationFunctionType.Square,
                accum_out=st[:, col + nb : col + nb + 1],
            )

        def emit_vec_piece(g, bidx, nb, base, mbase):
            # trailing piece fully evacuated on DVE (max + bn_stats; sumsq is
            # recovered on host from count/mean/var of even/odd interleaves)
            # into the shared final tile
            bank, _ = GROUPS[g]
            L = LS[bank]
            ps = run_mms(g, bank, bidx // 4, bidx % 4, nb, f"v{bidx}")
            pv = ps[:].rearrange("p (b t) -> p b t", t=128)[:, :, :L]
            nc.vector.tensor_reduce(
                stf[:, base : base + nb], pv, axis=mybir.AxisListType.X,
                op=mybir.AluOpType.max,
            )
            for j in range(nb):  # HW BNStats emits exactly 6 elems/partition
                nc.vector.bn_stats(
                    stf[:, base + nb + 6 * j : base + nb + 6 * (j + 1)],
                    pv[:, j, :],
                )
            if need_min:
                nc.vector.tensor_reduce(
                    stf[:, mbase : mbase + nb], pv, axis=mybir.AxisListType.X,
                    op=mybir.AluOpType.min,
                )

        def dma_bulk(g):
            w = STW if g < 5 else 15
            nc.sync.dma_start(stats_d[:, STW * g : STW * g + w], st_sb[g][:, 0:w])
            if need_min:
                nmin = 16 if g < 5 else 12
                nc.sync.dma_start(
                    stats_d[:, _min_base(g) : _min_base(g) + nmin],
                    st_sb[g][:, STW : STW + nmin],
                )

        # group 0: phased accumulation (pair0 taps as soon as its halves
        # land, pair1 taps later; PSUM groups stay open in between)
        bank0 = GROUPS[0][0]
        t0 = _group_tiles(bank0)
        half = len(t0) // 2
        g0_ps = [
            psum_pool.tile([128, 512], mybir.dt.float32, tag="ps", name=f"ps0_{q}")
            for q in range(4)
        ]
        for phase, qs in ((0, (0, 1)), (0, (2, 3)), (1, (0, 1)), (1, (2, 3))):
            tiles = t0[:half] if phase == 0 else t0[half:]
            for q in qs:
                for j, (ccp, kk) in enumerate(tiles):
                    i = phase * half + j
                    lhs = wt_sb[0][:, i * 256 : (i + 1) * 256].rearrange(
                        "p (c f) -> p c f", c=2
                    )
                    nc.tensor.matmul(
                        g0_ps[q][:], lhs, rhs_ap(ccp, q, kk, 0, 4),
                        start=(phase == 0 and j == 0),
                        stop=(phase == 1 and j == half - 1), perf_mode=DR,
                    )
                if phase == 1:
                    col, nb = PIECES_FULL[q]
                    st = st_sb[0]
                    L = LS[bank0]
                    pv = g0_ps[q][:].rearrange("p (b t) -> p b t", t=128)[:, :, :L]
                    nc.vector.tensor_reduce(
                        st[:, col : col + nb], pv, axis=mybir.AxisListType.X,
                        op=mybir.AluOpType.max,
                    )
                    if need_min:
                        nc.vector.tensor_reduce(
                            st[:, STW + q * 4 : STW + q * 4 + nb], pv,
                            axis=mybir.AxisListType.X, op=mybir.AluOpType.min,
                        )
                    scr = scr_pool.tile([128, 512], mybir.dt.float32, tag="scr")
                    scr_v = scr[:, : nb * L].rearrange("p (b t) -> p b t", t=L)
                    nc.scalar.activation(
                        scr_v, pv,
                        mybir.ActivationFunctionType.Square,
                        accum_out=st[:, col + nb : col + nb + 1],
                    )
        dma_bulk(0)

        for g in range(1, 5):
            bidx = 0
            for col, nb in PIECES_FULL:
                emit_piece(g, col, nb, bidx)
                bidx += nb
            dma_bulk(g)
        bidx = 0
        for col, nb in PIECES_BULK[:3]:
            emit_piece(5, col, nb, bidx)
            bidx += nb
        dma_bulk(5)
        emit_vec_piece(5, 12, 2, 0, 28)
        emit_vec_piece(5, 14, 1, 14, 30)
        emit_vec_piece(5, 15, 1, 21, 31)
        nc.sync.dma_start(
            stats_d[:, 120 : 152 if need_min else 148],
            stf[:, 0 : 32 if need_min else 28],
        )

    nc.compile()
    return nc


def _get_compiled(need_min):
    key = ("nc", need_min)
    if key not in _CACHE:
        _CACHE[key] = _build_bass(need_min)
    return _CACHE[key]


def _maybe_enable_trace():
    if os.environ.get("KERNEL_TRACE") != "1":
        return False
    try:
        import sys, types

        if "antenv.axon_hooks" not in sys.modules:
            mod = types.ModuleType("antenv.axon_hooks")
            _h = {"hook": None}
            mod.set_axon_ntff_profile_hook = lambda h: _h.__setitem__("hook", h)
            mod.get_axon_ntff_profile_hook = lambda: _h["hook"]
            sys.modules["antenv.axon_hooks"] = mod
            import antenv

            antenv.axon_hooks = mod
            from trn_agent_boot.trn_boot import _ntff_profile_via_ctypes

            mod.set_axon_ntff_profile_hook(
                _ntff_profile_via_ctypes("/opt/axon/libaxon_pjrt.so")
            )
        import concourse.bass_utils as bu

        bu.upload_artifacts = lambda tmpdir: tmpdir
        return True
    except Exception:
        return False


def _q8(a, sc):
    return np.clip(np.asarray(a, dtype=np.float32) * sc, -240.0, 240.0).astype(F8)


def kernel(
    x, emb_w,
    conv_w0, conv_b0, bn_g0, bn_b0,
    conv_w1, conv_b1, bn_g1, bn_b1,
    conv_w2, conv_b2, bn_g2, bn_b2,
    fc1_w, fc1_b, bn1_g, bn1_b, fc2_w, fc2_b,
):
    global _LAST_RESULTS
    from concourse.bass_utils import run_bass_kernel_spmd

    x = np.asarray(x, dtype=np.float32)
    emb_w = np.asarray(emb_w, dtype=np.float32)
    conv_ws = [np.asarray(w, dtype=np.float32) for w in (conv_w0, conv_w1, conv_w2)]
    bn_gs = [np.asarray(v, dtype=np.float64) for v in (bn_g0, bn_g1, bn_g2)]
    bn_bs = [np.asarray(v, dtype=np.float64) for v in (bn_b0, bn_b1, bn_b2)]
    need_min = bool((np.concatenate(bn_gs) < 0.0).any())

    # ---- host: embedding (x is one-hot in practice; dense matmul is exact) ----
    e = x.reshape(-1, V) @ emb_w                       # [B*S*W, E]
    e = e.reshape(B, S, CIN)                           # [B, S, 512]
    embT = np.ascontiguousarray(e.transpose(2, 0, 1))  # [512, B, S]
    emb8 = _q8(embT, SC_A)                             # [512, B, 128]

    # ---- pack device inputs ----
    ntiles = _weight_tile_count()
    wts = np.empty((128, ntiles * 256), dtype=F8)
    i = 0
    for bank, fc in GROUPS:
        cwq = _q8(conv_ws[bank], SC_W)                 # [256, 512, k]
        for ccp, kk in _group_tiles(bank):
            blk = cwq[fc * 128 : (fc + 1) * 128,
                      2 * ccp * 128 : (2 * ccp + 2) * 128, kk]  # [f, 2*128]
            # target [p, c*128 + f] = blk[f, c*128 + p]
            wts[:, i * 256 : (i + 1) * 256] = (
                blk.reshape(128, 2, 128).transpose(2, 1, 0).reshape(128, 256)
            )
            i += 1

    # emb8 viewed [pair, c, p, batch, t]
    ev = emb8.reshape(NPAIR, 2, 128, B, S)
    in_maps = []
    for c in range(NCORES):
        v = ev[:, :, :, c * BL : (c + 1) * BL, :].reshape(NPAIR, 2, 128, 2, 8, S)
        tmp = np.zeros((NPAIR, 128, 2, 2, XHP), dtype=F8)
        # [pair, c2, p, h, b, t] -> [pair, p, h, c2, (b t)]
        tmp[:, :, :, :, :XH] = v.transpose(0, 2, 3, 1, 4, 5).reshape(
            NPAIR, 128, 2, 2, XH
        )
        in_maps.append({"emb": tmp.reshape(NPAIR, 128, EMB_FREE), "wts": wts})

    nc = _get_compiled(need_min)
    trace = _maybe_enable_trace()
    res = run_bass_kernel_spmd(
        nc, in_maps, core_ids=list(range(NCORES)), trace=trace,
        tmpdir=os.environ.get("KERNEL_TRACE_DIR") or None,
    )
    _LAST_RESULTS = res

    # ---- host: combine stats -> BN -> pooled -> fc head (float64) ----
    FT = sum(FILTERS)  # 768
    inv = 1.0 / (SC_A * SC_W)
    cmax = np.empty((FT, B), dtype=np.float64)
    cmin = np.empty((FT, B), dtype=np.float64) if need_min else None
    sumsq = np.zeros(FT, dtype=np.float64)
    for c in range(NCORES):
        stats = res.results[c]["stats"].astype(np.float64)  # [128, nstat]
        for g, (bank, fc) in enumerate(GROUPS):
            ch = bank * 256 + fc * 128
            sl = slice(ch, ch + 128)
            pieces = PIECES_FULL if g < 5 else PIECES_FULL[:3]
            bidx = 0
            for col, nb in pieces:
                bs = slice(c * BL + bidx, c * BL + bidx + nb)
                cmax[sl, bs] = stats[:, STW * g + col : STW * g + col + nb] * inv
                sumsq[sl] += stats[:, STW * g + col + nb] * inv * inv
                if need_min:
                    mb = _min_base(g)
                    cmin[sl, bs] = stats[:, mb + bidx : mb + bidx + nb] * inv
                bidx += nb
            if g == 5:  # trailing 2+1+1 pieces: [max x nb, bn_stats x 6nb]
                for bidx, nb, base, mbase in ((12, 2, 0, 28), (14, 1, 14, 30),
                                              (15, 1, 21, 31)):
                    bs = slice(c * BL + bidx, c * BL + bidx + nb)
                    b0 = 120 + base
                    cmax[sl, bs] = stats[:, b0 : b0 + nb] * inv
                    bn = stats[:, b0 + nb : b0 + nb + 6 * nb].reshape(128, nb, 6)
                    sq = (bn[:, :, 2] + bn[:, :, 0] * bn[:, :, 1] ** 2
                          + bn[:, :, 5] + bn[:, :, 3] * bn[:, :, 4] ** 2)
                    sumsq[sl] += sq.sum(axis=1) * inv * inv
                    if need_min:
                        cmin[sl, bs] = stats[:, 120 + mbase : 120 + mbase + nb] * inv

    # channel means via the factorized sum (exact: sum_t conv = w . window-sums)
    embT64 = embT.astype(np.float64)
    st_sum = embT64.sum(axis=1)                        # [512, S] summed over batch
    cum = np.concatenate(
        [np.zeros((CIN, 1)), np.cumsum(st_sum, axis=1)], axis=1
    )                                                  # [512, S+1]
    mean = np.empty(FT, dtype=np.float64)
    for bank in range(3):
        k, L = KS[bank], LS[bank]
        cw = conv_ws[bank].astype(np.float64)          # [256, 512, k]
        hs = np.stack([cum[:, kk + L] - cum[:, kk] for kk in range(k)], axis=1)
        mean[bank * 256 : (bank + 1) * 256] = (
            np.einsum("fck,ck->f", cw, hs) / (B * L)
        )

    counts = np.repeat([B * L for L in LS], FILTERS)
    var = sumsq / counts - mean * mean
    g_all = np.concatenate(bn_gs)
    b_all = np.concatenate(bn_bs)
    s = g_all / np.sqrt(var + EPS)
    shift = b_all - mean * s
    M = np.where(s[:, None] >= 0.0, cmax, cmin if need_min else cmax)  # [768, B]
    pooled = np.maximum(s[:, None] * M + shift[:, None], 0.0).T  # [B, 768]

    z = pooled @ np.asarray(fc1_w, dtype=np.float64) + np.asarray(
        fc1_b, dtype=np.float64
    )
    mu = z.mean(axis=0, keepdims=True)
    vz = np.square(z - mu).mean(axis=0, keepdims=True)
    z = (z - mu) / np.sqrt(vz + EPS) * np.asarray(
        bn1_g, dtype=np.float64
    ) + np.asarray(bn1_b, dtype=np.float64)
    z = np.maximum(z, 0.0)
    logits = z @ np.asarray(fc2_w, dtype=np.float64) + np.asarray(
        fc2_b, dtype=np.float64
    )
    logits -= logits.max(axis=1, keepdims=True)
    p = np.exp(logits)
    p /= p.sum(axis=1, keepdims=True)
    return p.astype(np.float32)
